# revision 30
# baseline (speedup 1.0000x reference)
"""ContextQueryAttention (BiDAF-style) Trainium2 Bass kernel.

Full inputs -> full output; internally data-parallel over batch across 8
NeuronCores (4 batches per core).

Per-batch math (b dropped; C:[d,t], Q:[d,j], d=512, t=1024, j=128):
  H = C^T, U = Q^T
  S[t,j]  = hbias[t] + ubias[j] + sum_d w_hu[d]*C[d,t]*Q[d,j]
  S_q     = softmax_j(mask(S, mask_Q))         # masked -> -1e30
  S_c     = softmax_t(mask(S, mask_C))
  A       = S_q @ U                            # (t,d)
  q2c     = S_c^T @ H                          # (j,d)
  Bmat    = S_q @ q2c                          # (t,d)
  out     = [H; A; H*A; H*Bmat] as (4d, t)

Layout strategy: everything "feature-on-partitions", S^T [j,t] on PE with
hbias folded into the stationary weights (Qw = Q*w_hu + w_h). softmax_j
(partition dim) is done WITHOUT transposes via log-domain normalization:
  e1   = exp(mq*S^T + addc - 100)            (masked-Q, unnormalized)
  cs   = ones^T @ e1                          (PE column sum, [1,t])
  psS += -ones x ln(cs)                      (rank-1, PE)
  SqT  = exp(mq*S^T + addc - 100)            (now exactly softmax_j)
softmax_t is free-dim in the S^T layout with constant max subtraction.
mask_C applied additively via rank-1 fp32r matmul of (m-1)*1e30.
A^T / q2c / Bmat^T as fp32r matmuls (1 cyc/row); outputs written
[C; A^T; C*A^T; C*B^T] in natural d-on-partition layout; H block is a
direct DRAM->DRAM DMA.

DMA scheduling: inputs for batch b+1 are issued on the SP HWDGE queue
BEFORE batch b's compute so the DMA engines stream continuously
(input prefetch fills the S/softmax phase, outputs fill the tail);
A^T outputs go out on the Activation HWDGE queue so a gated store never
head-of-line-blocks the input stream.
"""

import numpy as np

import concourse.bass as bass
import concourse.tile as tile
from concourse import bacc, mybir
from concourse import bass_utils
from concourse.masks import make_identity

F32 = mybir.dt.float32
F32R = mybir.dt.float32r
BF16 = mybir.dt.bfloat16
I32 = mybir.dt.int32

B, T, J, D = 32, 1024, 128, 512
NCORES = 8
BPC = B // NCORES  # batches per core
MHAT = 100.0  # fixed max-subtraction constant for both softmaxes
NCH = D // 128  # 4 d-chunks
NTC = T // 128  # 8 t-chunks

DEBUG = False

AF = mybir.ActivationFunctionType
ALU = mybir.AluOpType


def _emit_loads(nc, pools, consts, aps, b, mask_C):
    """Input DMAs for batch b (SP HWDGE queue)."""
    (cin, qin, mid, outp, psS_pool, tp_pool, mm_pool, sm, crp) = pools
    (C, Q, out) = aps
    Qa = qin.tile([128, NCH * J], F32, tag="qt")
    nc.sync.dma_start(Qa[:].rearrange("p (c j) -> p c j", c=NCH),
                  Q[b].rearrange("(c p) j -> p c j", p=128))
    Qt = [Qa[:, J * i:J * (i + 1)] for i in range(NCH)]
    mc8 = qin.tile([8, 128], I32, tag="mc8")
    nc.sync.dma_start(mc8[:], mask_C[b].rearrange("(c p) -> c p", c=NTC))
    Ct = []
    for i in range(NCH):
        t = cin.tile([128, T], F32, tag="ct")
        nc.sync.dma_start(t[:], C[b, 128 * i:128 * (i + 1), :])
        Ct.append(t)
    return dict(Ct=Ct, Qt=Qt, mc8=mc8)


def _emit_round(nc, pools, consts, L, b):
    """One-batch-ahead prep: fp32r rounding of C (Pool, idle during the
    previous batch's softmax), Qw / mask scalars (DVE), ubias (PE).
    Emitting these a batch early keeps them out of the next batch's
    critical path."""
    (cin, qin, mid, outp, psS_pool, tp_pool, mm_pool, sm, crp) = pools
    (identity, id_r, w_col, ones_col_r, ones_r32, neg_r32, neg8_r32,
     pos8_r32, mqf_all, mhat_neg, tiny) = consts
    Qt = L["Qt"]
    Cr = []
    for i in range(NCH):
        t = crp.tile([128, T], F32R, tag="cr", name=f"cr{b}_{i}")
        if b == 0 and i % 2 == 1:
            nc.vector.tensor_copy(t[:], L["Ct"][i][:])
        else:
            nc.gpsimd.tensor_copy(t[:], L["Ct"][i][:])
        Cr.append(t)
    L["Cr"] = Cr
    # mask_C as per-partition columns in the H (t-on-partitions) layout:
    # contiguous [8,128] load, f32 convert, PE transpose into spare
    # columns of the ub_ps bank (no extra PSUM footprint)
    mc8f = sm.tile([8, 128], F32, tag="mc8f", bufs=4, name=f"mc8f{b}")
    nc.vector.tensor_copy(mc8f[:], L["mc8"][:])
    # Qw = Q * w_hu + w_h: adding w_h[d] to every column folds
    # hbias[t] = sum_d w_h[d] C[d,t] into the S^T core matmul output.
    Qw = []
    for i in range(NCH):
        t = sm.tile([128, J], F32R, tag="qw", bufs=8, name=f"qw{b}_{i}")
        nc.vector.tensor_scalar_mul(t[:], Qt[i], w_col[:, 8 + i:9 + i])
        nc.vector.tensor_scalar_add(t[:], t[:], w_col[:, i:i + 1])
        Qw.append(t)
    L["Qw"] = Qw
    # ubias[j] = sum_d w_u[d] Q[d,j] : N=1 matmuls -> [j,1]
    ub_ps = psS_pool.tile([128, 512], F32, tag="psS", bufs=2,
                          name=f"ub{b}")
    for i in range(NCH):
        nc.tensor.matmul(ub_ps[:, 0:1], Qt[i], w_col[:, 4 + i:5 + i],
                         start=(i == 0), stop=(i == NCH - 1))
    nc.tensor.transpose(ub_ps[:, 8:8 + NTC], mc8f[:], identity[0:8, 0:8])
    mcc = sm.tile([128, NTC], F32, tag="mcc", bufs=4, name=f"mcc{b}")
    nc.vector.tensor_copy(mcc[:], ub_ps[:, 8:8 + NTC])
    L["mcc"] = mcc
    # mask_Q: scale = mqf, bias = mqf*(ub - MHAT) + (mqf-1)*1e30
    mqf = mqf_all[:, b:b + 1]
    c1 = sm.tile([128, 1], F32, tag="c1")
    nc.vector.tensor_scalar(c1[:], mqf, 1.0e30, -1.0e30,
                            op0=ALU.mult, op1=ALU.add)
    c2 = sm.tile([128, 1], F32, tag="c2")
    nc.vector.tensor_scalar_add(c2[:], ub_ps[:, 0:1], -MHAT)
    nc.vector.tensor_mul(c2[:], c2[:], mqf)
    addc = sm.tile([128, 1], F32, tag="addc")
    nc.vector.tensor_add(addc[:], c1[:], c2[:])
    # 1/8-compressed variants for the range-finding pass
    mq8 = sm.tile([128, 1], F32, tag="mq8")
    nc.vector.tensor_scalar_mul(mq8[:], mqf, 0.125)
    addc8 = sm.tile([128, 1], F32, tag="addc8")
    nc.vector.tensor_scalar_mul(addc8[:], addc[:], 0.125)
    L["addc"] = addc
    L["mq8"] = mq8
    L["addc8"] = addc8


def _emit_batch(nc, tc, pools, consts, aps, b, L):
    """Emit compute + output DMAs for one batch."""
    (cin, qin, mid, outp, psS_pool, tp_pool, mm_pool, sm, crp) = pools
    (identity, id_r, w_col, ones_col_r, ones_r32, neg_r32, neg8_r32,
     pos8_r32, mqf_all, mhat_neg, tiny) = consts
    (C, Q, out) = aps
    Ct, Qt = L["Ct"], L["Qt"]
    Cr, Qw, mcc = L["Cr"], L["Qw"], L["mcc"]
    addc, mq8, addc8 = L["addc"], L["mq8"], L["addc8"]
    mqf = mqf_all[:, b:b + 1]

    # ---- S^T core into PSUM (one bank per t-half), then the two softmax
    # paths.  PE order interleaves the halves so PE works on half 1 while
    # ACT processes half 0. ----
    e1 = mid.tile([128, T], F32R, tag="sqt", bufs=2,
                  name=f"e1_{b}")
    SqT = mid.tile([128, T], F32R, tag="sqt", bufs=2,
                   name=f"sqt{b}")
    ecT = mid.tile([128, T], F32R, tag="ect")

    # H transposes double as PE filler work inside the ACT-bound S chain:
    # emitted between S-chain stages so PE has work while ACT runs.
    H_sb = [None] * NTC
    tH_done = [0]

    def emit_H_chunk():
        c = tH_done[0]
        if c >= NTC:
            return
        tH_done[0] += 1
        tH = tp_pool.tile([128, 512], F32, tag="tp", name=f"tH{b}_{c}")
        for i in range(NCH):
            nc.tensor.transpose(tH[:, 128 * i:128 * (i + 1)].bitcast(F32R),
                                Cr[i][:, 128 * c:128 * (c + 1)], id_r[:])
        hs = mid.tile([128, 512], F32R, tag="hsb", bufs=9,
                      name=f"hsb{b}_{c}")
        nc.vector.tensor_scalar_mul(hs[:], tH[:], mcc[:, c:c + 1])
        H_sb[c] = hs

    for h in range(2):
        sl = slice(512 * h, 512 * (h + 1))
        mlog = sm.tile([1, 512], F32R, tag="mlog", bufs=2,
                       name=f"mlog{b}_{h}")
        ln8 = sm.tile([1, 512], F32R, tag="ln8", bufs=2,
                      name=f"ln8{b}_{h}")
        psSh = psS_pool.tile([128, 512], F32, tag="psS", bufs=2,
                             name=f"psS{b}_{h}")
        for i in range(NCH):
            nc.tensor.matmul(psSh[:], Qw[i][:], Cr[i][:, sl],
                             start=(i == 0), stop=False)
        # pass 0 (range finder): exp((S+ub-100)/8); 8*ln of its column sum
        # is >= the column max of S+ub-100 (within +8*ln(128)), keeping
        # every later exp/ln safely in range.
        nc.scalar.activation(e1[:, sl], psSh[:], AF.Exp,
                             bias=addc8[:], scale=mq8[:])
        emit_H_chunk()  # PE filler while ACT runs
        pcs8 = psS_pool.tile([1, 512], F32, tag="cs", bufs=1,
                             name=f"cs8{b}_{h}")
        nc.tensor.matmul(pcs8[:], ones_col_r[:], e1[:, sl],
                         start=True, stop=True)
        nc.scalar.activation(ln8[:], pcs8[:], AF.Ln, bias=tiny[:])
        nc.tensor.matmul(psSh[:], neg8_r32[:], ln8[:],
                         start=False, stop=False)
        # pass 1: shifted masked-Q exp, values in (0, 1]
        nc.scalar.activation(e1[:, sl], psSh[:], AF.Exp,
                             bias=addc[:], scale=mqf)
        emit_H_chunk()  # PE filler while ACT does pass 1
        # column sum over j (partition dim) on PE -> [1, 512]
        pcs = psS_pool.tile([1, 512], F32, tag="cs", bufs=1,
                            name=f"cs{b}_{h}")
        nc.tensor.matmul(pcs[:], ones_col_r[:], e1[:, sl],
                         start=True, stop=True)
        # +ln(colsum) row; rank-1 subtract into psS
        nc.scalar.activation(mlog[:], pcs[:], AF.Ln, bias=tiny[:])
        emit_H_chunk()  # PE filler while ACT does the Ln
        nc.tensor.matmul(psSh[:], neg_r32[:], mlog[:],
                         start=False, stop=False)
        # pass 2: normalized softmax_j -> SqT
        nc.scalar.activation(SqT[:, sl], psSh[:], AF.Exp,
                             bias=addc[:], scale=mqf)
        emit_H_chunk()  # PE filler while ACT does pass 2
        # undo the shifts, add mask_C row
        nc.tensor.matmul(psSh[:], ones_r32[:], mlog[:],
                         start=False, stop=False)
        nc.tensor.matmul(psSh[:], pos8_r32[:], ln8[:],
                         start=False, stop=True)
        # pass 3: S_c exp with constant max subtraction (mask_C is applied
        # multiplicatively to H, so masked-t terms drop out of q2c and the
        # column sum below)
        nc.scalar.activation(ecT[:, sl], psSh[:], AF.Exp,
                             bias=mhat_neg[:], scale=1.0)
        emit_H_chunk()  # PE filler while ACT does pass 3

    # ---- remaining H = C^T tiles [t,d] -> fp32r SBUF ----
    while tH_done[0] < NTC:
        emit_H_chunk()

    # ---- Q^T [j,d] fp32r ----
    tQ = tp_pool.tile([128, 512], F32, tag="tp")
    for i in range(NCH):
        nc.tensor.transpose(tQ[:, 128 * i:128 * (i + 1)], Qt[i],
                            identity[:])
    QT = mid.tile([128, 512], F32R, tag="qT")
    nc.vector.tensor_copy(QT[:], tQ[:])

    # ---- e_c transposes -> [t,j] fp32r chunks ----
    te = [tp_pool.tile([128, 512], F32, tag="tp", name=f"te{b}_{k}")
          for k in range(2)]
    for c in range(NTC):
        nc.tensor.transpose(
            te[c // 4][:, 128 * (c % 4):128 * (c % 4 + 1)].bitcast(F32R),
            ecT[:, 128 * c:128 * (c + 1)], id_r[:])
    ec_sb = mid.tile([128, T], F32R, tag="ecsb")
    for k in range(2):
        nc.vector.tensor_copy(ec_sb[:, 512 * k:512 * (k + 1)], te[k][:])
    csJ = psS_pool.tile([128, 512], F32, tag="psS", bufs=2, name=f"csj{b}")
    for c in range(NTC):
        nc.tensor.matmul(csJ[:, 0:1],
                         ec_sb[:, 128 * c:128 * (c + 1)].bitcast(F32),
                         mcc[:, c:c + 1], start=(c == 0),
                         stop=(c == NTC - 1))
    rc = sm.tile([128, 1], F32, tag="rc")
    nc.vector.reciprocal(rc[:], csJ[:, 0:1])

    # ---- A path first (needs only QT+SqT): outputs start flowing while
    # the q2c chain (interleaved below) finishes for the B path ----
    A_sb = [outp.tile([128, T], F32, tag="asb", name=f"a{b}_{m}")
            for m in range(NCH)]
    HB = [outp.tile([128, T], F32, tag="hbsb", name=f"hb{b}_{m}")
          for m in range(NCH)]
    HA = [outp.tile([128, T], F32, tag="hasb", name=f"ha{b}_{m}")
          for m in range(NCH)]
    psq = psS_pool.tile([128, 512], F32, tag="psS", bufs=2,
                        name=f"psq{b}")
    q2c = mid.tile([128, 512], F32R, tag="q2c")
    for m in range(NCH):
        for h in range(2):
            sl = slice(512 * h, 512 * (h + 1))
            psA = mm_pool.tile([128, 512], F32, tag="mm")
            nc.tensor.matmul(psA[:], QT[:, 128 * m:128 * (m + 1)], SqT[:, sl],
                             start=True, stop=True)
            nc.scalar.activation(A_sb[m][:, sl], psA[:], AF.Copy)
            nc.sync.dma_start(
                out[b, D + 128 * m:D + 128 * (m + 1), sl],
                A_sb[m][:, sl])
        if m == 0:
            # q2c = (1/csum) * sum_c e_c[c].T @ H[c] -> [j,d] fp32r
            for c in range(NTC):
                nc.tensor.matmul(psq[:], ec_sb[:, 128 * c:128 * (c + 1)],
                                 H_sb[c][:], start=(c == 0),
                                 stop=(c == NTC - 1))
            nc.vector.tensor_scalar_mul(q2c[:], psq[:], rc[:])
        nc.gpsimd.tensor_mul(HA[m][:], Cr[m][:], A_sb[m][:])
        if m > 0:
            nc.sync.dma_start(
                out[b, 2 * D + 128 * (m - 1):2 * D + 128 * m, :],
                HA[m - 1][:])
    nc.sync.dma_start(out[b, 2 * D + 128 * (NCH - 1):2 * D + 128 * NCH, :],
                      HA[NCH - 1][:])
    # ---- B path: psB -> H*B fused evacuation, per-half stores ----
    for m in range(NCH):
        for h in range(2):
            sl = slice(512 * h, 512 * (h + 1))
            psB = mm_pool.tile([128, 512], F32, tag="mm")
            nc.tensor.matmul(psB[:], q2c[:, 128 * m:128 * (m + 1)], SqT[:, sl],
                             start=True, stop=True)
            nc.vector.tensor_mul(HB[m][:, sl], Cr[m][:, sl], psB[:])
            nc.sync.dma_start(
                out[b, 3 * D + 128 * m:3 * D + 128 * (m + 1), sl],
                HB[m][:, sl])

    if b == 0 and getattr(nc, "_dbg", None):
        dbg = nc._dbg
        nc.sync.dma_start(dbg["e1"], e1[:].bitcast(F32))
        nc.sync.dma_start(dbg["sqt"], SqT[:].bitcast(F32))
        nc.sync.dma_start(dbg["ect"], ecT[:].bitcast(F32))
        nc.sync.dma_start(dbg["mlog"], mlog[:].bitcast(F32))
        nc.sync.dma_start(dbg["csum"], csum[:])
        nc.sync.dma_start(dbg["rc"], rc[:])
        nc.sync.dma_start(dbg["addc"], addc[:])
        nc.sync.dma_start(dbg["q2c"], q2c[:].bitcast(F32))
        nc.sync.dma_start(dbg["qT"], QT[:].bitcast(F32))
        nc.sync.dma_start(dbg["ecsb"], ec_sb[:].bitcast(F32))
        nc.sync.dma_start(dbg["h0"], H_sb[0][:].bitcast(F32))
        nc.sync.dma_start(dbg["h1"], H_sb[1][:].bitcast(F32))
        nc.sync.dma_start(dbg["qw0"], Qw[0][:])



def _pin_act_table():
    """Pin activation-table selection to 'natural_log_exp_and_others' (the
    one set containing ln+exp+copy+identity together).  The greedy table
    chooser otherwise thrashes between the exp-only and ln-only sets,
    inserting a 1283ns LoadActFuncSet at every Exp<->Ln alternation.
    Entry positions are preserved so act_func_set_id stays a valid index
    into act_info.json."""
    import concourse.bacc as bacc_mod
    if getattr(bacc_mod, "_act_tables_pinned", False):
        return
    orig = bacc_mod.get_activation_tables

    def pinned(arch):
        return {name: (fns if name == "natural_log_exp_and_others" else set())
                for name, fns in orig(arch).items()}

    bacc_mod.get_activation_tables = pinned
    bacc_mod._act_tables_pinned = True


def _build():
    _pin_act_table()
    nc = bacc.Bacc("TRN2", target_bir_lowering=False, debug=False,
                   num_devices=NCORES)
    C = nc.dram_tensor("C", [BPC, D, T], F32, kind="ExternalInput").ap()
    Q = nc.dram_tensor("Q", [BPC, D, J], F32, kind="ExternalInput").ap()
    mask_C = nc.dram_tensor("mask_C", [BPC, T], I32, kind="ExternalInput").ap()
    mask_Q = nc.dram_tensor("mask_Q", [BPC, J], I32, kind="ExternalInput").ap()
    weight = nc.dram_tensor("weight", [3 * D], F32, kind="ExternalInput").ap()
    out = nc.dram_tensor("out", [BPC, 4 * D, T], F32,
                         kind="ExternalOutput").ap()
    dbg = {}
    if DEBUG:
        for nm, shp in [("e1", [128, T]), ("sqt", [128, T]), ("ect", [128, T]),
                        ("mlog", [1, T]), ("csum", [128, 1]), ("rc", [128, 1]),
                        ("addc", [128, 1]), ("q2c", [128, D]), ("qT", [128, D]),
                        ("ecsb", [128, T]), ("h0", [128, D]), ("h1", [128, D]),
                        ("qw0", [128, J])]:
            dbg[nm] = nc.dram_tensor(f"dbg_{nm}", shp, F32,
                                     kind="ExternalOutput").ap()
    nc._dbg = dbg

    with tile.TileContext(nc) as tc:
        import contextlib
        with contextlib.ExitStack() as ctx:
            const = ctx.enter_context(tc.tile_pool(name="const", bufs=1))
            cin = ctx.enter_context(tc.tile_pool(name="cin", bufs=8))
            crp = ctx.enter_context(tc.tile_pool(name="crp", bufs=11))
            qin = ctx.enter_context(tc.tile_pool(name="qin", bufs=4))
            mid = ctx.enter_context(tc.tile_pool(name="mid", bufs=2))
            outp = ctx.enter_context(tc.tile_pool(name="outp", bufs=4))
            sm = ctx.enter_context(tc.tile_pool(name="sm", bufs=4))
            psS_pool = ctx.enter_context(
                tc.tile_pool(name="psS", bufs=2, space="PSUM"))
            tp_pool = ctx.enter_context(
                tc.tile_pool(name="tp", bufs=2, space="PSUM"))
            mm_pool = ctx.enter_context(
                tc.tile_pool(name="mm", bufs=3, space="PSUM"))

            # ---- constants ----
            identity = const.tile([128, 128], F32, tag="identity")
            make_identity(nc, identity[:])
            id_r = const.tile([128, 128], F32R, tag="id_r")
            nc.vector.tensor_copy(id_r[:], identity[:])
            ones128 = const.tile([128, 1], F32, tag="ones128")
            nc.gpsimd.memset(ones128[:], 1.0)
            ones_col_r = const.tile([128, 1], F32R, tag="ones_col_r")
            nc.vector.tensor_copy(ones_col_r[:], ones128[:])
            onesr_f = const.tile([1, 128], F32, tag="onesr_f")
            nc.gpsimd.memset(onesr_f[:], 1.0)
            ones_r32 = const.tile([1, 128], F32R, tag="ones_r32")
            nc.vector.tensor_copy(ones_r32[:], onesr_f[:])
            neg_r32 = const.tile([1, 128], F32R, tag="neg_r32")
            nc.vector.tensor_scalar_mul(neg_r32[:], onesr_f[:], -1.0)
            neg8_r32 = const.tile([1, 128], F32R, tag="neg8_r32")
            nc.vector.tensor_scalar_mul(neg8_r32[:], onesr_f[:], -8.0)
            pos8_r32 = const.tile([1, 128], F32R, tag="pos8_r32")
            nc.vector.tensor_scalar_mul(pos8_r32[:], onesr_f[:], 8.0)
            # weight -> [128, 12]: cols g*4+c hold weight[g*512 + c*128 + p]
            w_col = const.tile([128, 12], F32, tag="w_col")
            nc.sync.dma_start(
                w_col[:], weight.rearrange("(g c p) -> p (g c)", p=128, c=4))
            # mask_Q -> [128, BPC] fp32
            mqi = const.tile([128, BPC], I32, tag="mqi")
            nc.sync.dma_start(mqi[:], mask_Q.rearrange("b j -> j b"))
            mqf_all = const.tile([128, BPC], F32, tag="mqf")
            nc.vector.tensor_copy(mqf_all[:], mqi[:])
            mhat_neg = const.tile([128, 1], F32, tag="mhat")
            nc.gpsimd.memset(mhat_neg[:], -MHAT)
            tiny = const.tile([1, 1], F32, tag="tiny")
            nc.gpsimd.memset(tiny[:], 1e-30)

            consts = (identity, id_r, w_col, ones_col_r, ones_r32,
                      neg_r32, neg8_r32, pos8_r32, mqf_all, mhat_neg,
                      tiny)
            pools = (cin, qin, mid, outp, psS_pool, tp_pool, mm_pool, sm,
                     crp)
            aps = (C, Q, out)

            # software pipeline: batch b+1's input loads are issued on the
            # SP queue BEFORE batch b's compute/stores so the DMA engines
            # never sit idle during the softmax phase.
            L = [_emit_loads(nc, pools, consts, aps, b, mask_C)
                 for b in range(BPC)]
            _emit_round(nc, pools, consts, L[0], 0)
            for b in range(BPC):
                if b + 1 < BPC:
                    _emit_round(nc, pools, consts, L[b + 1], b + 1)
                # H block: direct DRAM->DRAM, no dependencies (the last
                # one is hoisted a cycle early to fill the batch-2 bubble)
                if b < BPC - 1:
                    nc.sync.dma_start(out[b, 0:D, :], C[b, :, :])
                    if b == BPC - 2:
                        nc.sync.dma_start(out[b + 1, 0:D, :], C[b + 1, :, :])
                _emit_batch(nc, tc, pools, consts, aps, b, L[b])
                L[b] = None

    nc.compile()
    return nc


_NC_CACHE = None


def _get_nc():
    global _NC_CACHE
    if _NC_CACHE is None:
        _NC_CACHE = _build()
    return _NC_CACHE


def kernel(C, Q, mask_C, mask_Q, weight):
    nc = _get_nc()
    C = np.ascontiguousarray(C, dtype=np.float32)
    Q = np.ascontiguousarray(Q, dtype=np.float32)
    mask_C = np.ascontiguousarray(mask_C, dtype=np.int32)
    mask_Q = np.ascontiguousarray(mask_Q, dtype=np.int32)
    weight = np.ascontiguousarray(weight, dtype=np.float32)
    in_maps = []
    for c in range(NCORES):
        sl = slice(BPC * c, BPC * (c + 1))
        in_maps.append({
            "C": C[sl], "Q": Q[sl], "mask_C": mask_C[sl],
            "mask_Q": mask_Q[sl], "weight": weight,
        })
    res = bass_utils.run_bass_kernel_spmd(nc, in_maps,
                                          core_ids=list(range(NCORES)))
    return np.concatenate([res.results[c]["out"] for c in range(NCORES)],
                          axis=0)


# revision 31
# speedup vs baseline: 1.0004x; 1.0004x over previous
"""ContextQueryAttention (BiDAF-style) Trainium2 Bass kernel.

Full inputs -> full output; internally data-parallel over batch across 8
NeuronCores (4 batches per core).

Per-batch math (b dropped; C:[d,t], Q:[d,j], d=512, t=1024, j=128):
  H = C^T, U = Q^T
  S[t,j]  = hbias[t] + ubias[j] + sum_d w_hu[d]*C[d,t]*Q[d,j]
  S_q     = softmax_j(mask(S, mask_Q))         # masked -> -1e30
  S_c     = softmax_t(mask(S, mask_C))
  A       = S_q @ U                            # (t,d)
  q2c     = S_c^T @ H                          # (j,d)
  Bmat    = S_q @ q2c                          # (t,d)
  out     = [H; A; H*A; H*Bmat] as (4d, t)

Layout strategy: everything "feature-on-partitions", S^T [j,t] on PE with
hbias folded into the stationary weights (Qw = Q*w_hu + w_h). softmax_j
(partition dim) is done WITHOUT transposes via log-domain normalization:
  e1   = exp(mq*S^T + addc - 100)            (masked-Q, unnormalized)
  cs   = ones^T @ e1                          (PE column sum, [1,t])
  psS += -ones x ln(cs)                      (rank-1, PE)
  SqT  = exp(mq*S^T + addc - 100)            (now exactly softmax_j)
softmax_t is free-dim in the S^T layout with constant max subtraction.
mask_C applied additively via rank-1 fp32r matmul of (m-1)*1e30.
A^T / q2c / Bmat^T as fp32r matmuls (1 cyc/row); outputs written
[C; A^T; C*A^T; C*B^T] in natural d-on-partition layout; H block is a
direct DRAM->DRAM DMA.

DMA scheduling: inputs for batch b+1 are issued on the SP HWDGE queue
BEFORE batch b's compute so the DMA engines stream continuously
(input prefetch fills the S/softmax phase, outputs fill the tail);
A^T outputs go out on the Activation HWDGE queue so a gated store never
head-of-line-blocks the input stream.
"""

import numpy as np

import concourse.bass as bass
import concourse.tile as tile
from concourse import bacc, mybir
from concourse import bass_utils
from concourse.masks import make_identity

F32 = mybir.dt.float32
F32R = mybir.dt.float32r
BF16 = mybir.dt.bfloat16
I32 = mybir.dt.int32

B, T, J, D = 32, 1024, 128, 512
NCORES = 8
BPC = B // NCORES  # batches per core
MHAT = 100.0  # fixed max-subtraction constant for both softmaxes
NCH = D // 128  # 4 d-chunks
NTC = T // 128  # 8 t-chunks

DEBUG = False

AF = mybir.ActivationFunctionType
ALU = mybir.AluOpType


def _emit_loads(nc, pools, consts, aps, b, mask_C):
    """Input DMAs for batch b (SP HWDGE queue)."""
    (cin, qin, mid, outp, psS_pool, tp_pool, mm_pool, sm, crp) = pools
    (C, Q, out) = aps
    Qa = qin.tile([128, NCH * J], F32, tag="qt")
    nc.sync.dma_start(Qa[:].rearrange("p (c j) -> p c j", c=NCH),
                  Q[b].rearrange("(c p) j -> p c j", p=128))
    Qt = [Qa[:, J * i:J * (i + 1)] for i in range(NCH)]
    mci = qin.tile([128, NTC], I32, tag="mci")
    nc.sync.dma_start(mci[:], mask_C[b].rearrange("(c p) -> p c", p=128))
    Ct = []
    for i in range(NCH):
        t = cin.tile([128, T], F32, tag="ct")
        nc.sync.dma_start(t[:], C[b, 128 * i:128 * (i + 1), :])
        Ct.append(t)
    return dict(Ct=Ct, Qt=Qt, mci=mci)


def _emit_round(nc, pools, consts, L, b):
    """One-batch-ahead prep: fp32r rounding of C (Pool, idle during the
    previous batch's softmax), Qw / mask scalars (DVE), ubias (PE).
    Emitting these a batch early keeps them out of the next batch's
    critical path."""
    (cin, qin, mid, outp, psS_pool, tp_pool, mm_pool, sm, crp) = pools
    (identity, id_r, w_col, ones_col_r, ones_r32, neg_r32, neg8_r32,
     pos8_r32, mqf_all, mhat_neg, tiny) = consts
    Qt = L["Qt"]
    Cr = []
    for i in range(NCH):
        t = crp.tile([128, T], F32R, tag="cr", name=f"cr{b}_{i}")
        if b == 0 and i % 2 == 1:
            nc.vector.tensor_copy(t[:], L["Ct"][i][:])
        else:
            nc.gpsimd.tensor_copy(t[:], L["Ct"][i][:])
        Cr.append(t)
    L["Cr"] = Cr
    # mask_C as per-partition columns in the H (t-on-partitions) layout
    mcc = sm.tile([128, NTC], F32, tag="mcc", bufs=4, name=f"mcc{b}")
    nc.vector.tensor_copy(mcc[:], L["mci"][:])
    L["mcc"] = mcc
    # Qw = Q * w_hu + w_h: adding w_h[d] to every column folds
    # hbias[t] = sum_d w_h[d] C[d,t] into the S^T core matmul output.
    Qw = []
    for i in range(NCH):
        t = sm.tile([128, J], F32R, tag="qw", bufs=8, name=f"qw{b}_{i}")
        nc.vector.tensor_scalar_mul(t[:], Qt[i], w_col[:, 8 + i:9 + i])
        nc.vector.tensor_scalar_add(t[:], t[:], w_col[:, i:i + 1])
        Qw.append(t)
    L["Qw"] = Qw
    # ubias[j] = sum_d w_u[d] Q[d,j] : N=1 matmuls -> [j,1]
    ub_ps = psS_pool.tile([128, 512], F32, tag="psS", bufs=2,
                          name=f"ub{b}")
    for i in range(NCH):
        nc.tensor.matmul(ub_ps[:, 0:1], Qt[i], w_col[:, 4 + i:5 + i],
                         start=(i == 0), stop=(i == NCH - 1))
    # mask_Q: scale = mqf, bias = mqf*(ub - MHAT) + (mqf-1)*1e30
    mqf = mqf_all[:, b:b + 1]
    c1 = sm.tile([128, 1], F32, tag="c1")
    nc.vector.tensor_scalar(c1[:], mqf, 1.0e30, -1.0e30,
                            op0=ALU.mult, op1=ALU.add)
    c2 = sm.tile([128, 1], F32, tag="c2")
    nc.vector.tensor_scalar_add(c2[:], ub_ps[:, 0:1], -MHAT)
    nc.vector.tensor_mul(c2[:], c2[:], mqf)
    addc = sm.tile([128, 1], F32, tag="addc")
    nc.vector.tensor_add(addc[:], c1[:], c2[:])
    # 1/8-compressed variants for the range-finding pass
    mq8 = sm.tile([128, 1], F32, tag="mq8")
    nc.vector.tensor_scalar_mul(mq8[:], mqf, 0.125)
    addc8 = sm.tile([128, 1], F32, tag="addc8")
    nc.vector.tensor_scalar_mul(addc8[:], addc[:], 0.125)
    L["addc"] = addc
    L["mq8"] = mq8
    L["addc8"] = addc8


def _emit_batch(nc, tc, pools, consts, aps, b, L):
    """Emit compute + output DMAs for one batch."""
    (cin, qin, mid, outp, psS_pool, tp_pool, mm_pool, sm, crp) = pools
    (identity, id_r, w_col, ones_col_r, ones_r32, neg_r32, neg8_r32,
     pos8_r32, mqf_all, mhat_neg, tiny) = consts
    (C, Q, out) = aps
    Ct, Qt = L["Ct"], L["Qt"]
    Cr, Qw, mcc = L["Cr"], L["Qw"], L["mcc"]
    addc, mq8, addc8 = L["addc"], L["mq8"], L["addc8"]
    mqf = mqf_all[:, b:b + 1]

    # ---- S^T core into PSUM (one bank per t-half), then the two softmax
    # paths.  PE order interleaves the halves so PE works on half 1 while
    # ACT processes half 0. ----
    e1 = mid.tile([128, T], F32R, tag="sqt", bufs=2,
                  name=f"e1_{b}")
    SqT = mid.tile([128, T], F32R, tag="sqt", bufs=2,
                   name=f"sqt{b}")
    ecT = mid.tile([128, T], F32R, tag="ect")

    # ---- Q^T [j,d] fp32r up front (depends only on the Q load) so the
    # A matmuls can fire the moment pass 2 finishes ----
    tQ = tp_pool.tile([128, 512], F32, tag="tp", name=f"tQ{b}")
    for i in range(NCH):
        nc.tensor.transpose(tQ[:, 128 * i:128 * (i + 1)], Qt[i],
                            identity[:])
    QT = mid.tile([128, 512], F32R, tag="qT")
    nc.vector.tensor_copy(QT[:], tQ[:])

    # H transposes double as PE filler work inside the ACT-bound S chain:
    # emitted between S-chain stages so PE has work while ACT runs.
    H_sb = [None] * NTC
    tH_done = [0]

    def emit_H_chunk():
        c = tH_done[0]
        if c >= NTC:
            return
        tH_done[0] += 1
        tH = tp_pool.tile([128, 512], F32, tag="tp", name=f"tH{b}_{c}")
        for i in range(NCH):
            nc.tensor.transpose(tH[:, 128 * i:128 * (i + 1)].bitcast(F32R),
                                Cr[i][:, 128 * c:128 * (c + 1)], id_r[:])
        hs = mid.tile([128, 512], F32R, tag="hsb", bufs=9,
                      name=f"hsb{b}_{c}")
        nc.vector.tensor_scalar_mul(hs[:], tH[:], mcc[:, c:c + 1])
        H_sb[c] = hs

    for h in range(2):
        sl = slice(512 * h, 512 * (h + 1))
        mlog = sm.tile([1, 512], F32R, tag="mlog", bufs=2,
                       name=f"mlog{b}_{h}")
        ln8 = sm.tile([1, 512], F32R, tag="ln8", bufs=2,
                      name=f"ln8{b}_{h}")
        psSh = psS_pool.tile([128, 512], F32, tag="psS", bufs=2,
                             name=f"psS{b}_{h}")
        for i in range(NCH):
            nc.tensor.matmul(psSh[:], Qw[i][:], Cr[i][:, sl],
                             start=(i == 0), stop=False)
        # pass 0 (range finder): exp((S+ub-100)/8); 8*ln of its column sum
        # is >= the column max of S+ub-100 (within +8*ln(128)), keeping
        # every later exp/ln safely in range.
        nc.scalar.activation(e1[:, sl], psSh[:], AF.Exp,
                             bias=addc8[:], scale=mq8[:])
        emit_H_chunk()  # PE filler while ACT runs
        pcs8 = psS_pool.tile([1, 512], F32, tag="cs", bufs=1,
                             name=f"cs8{b}_{h}")
        nc.tensor.matmul(pcs8[:], ones_col_r[:], e1[:, sl],
                         start=True, stop=True)
        nc.scalar.activation(ln8[:], pcs8[:], AF.Ln, bias=tiny[:])
        nc.tensor.matmul(psSh[:], neg8_r32[:], ln8[:],
                         start=False, stop=False)
        # pass 1: shifted masked-Q exp, values in (0, 1]
        nc.scalar.activation(e1[:, sl], psSh[:], AF.Exp,
                             bias=addc[:], scale=mqf)
        emit_H_chunk()  # PE filler while ACT does pass 1
        # column sum over j (partition dim) on PE -> [1, 512]
        pcs = psS_pool.tile([1, 512], F32, tag="cs", bufs=1,
                            name=f"cs{b}_{h}")
        nc.tensor.matmul(pcs[:], ones_col_r[:], e1[:, sl],
                         start=True, stop=True)
        # +ln(colsum) row; rank-1 subtract into psS
        nc.scalar.activation(mlog[:], pcs[:], AF.Ln, bias=tiny[:])
        emit_H_chunk()  # PE filler while ACT does the Ln
        nc.tensor.matmul(psSh[:], neg_r32[:], mlog[:],
                         start=False, stop=False)
        # pass 2: normalized softmax_j -> SqT
        nc.scalar.activation(SqT[:, sl], psSh[:], AF.Exp,
                             bias=addc[:], scale=mqf)
        emit_H_chunk()  # PE filler while ACT does pass 2
        # undo the shifts, add mask_C row
        nc.tensor.matmul(psSh[:], ones_r32[:], mlog[:],
                         start=False, stop=False)
        nc.tensor.matmul(psSh[:], pos8_r32[:], ln8[:],
                         start=False, stop=True)
        # pass 3: S_c exp with constant max subtraction (mask_C is applied
        # multiplicatively to H, so masked-t terms drop out of q2c and the
        # column sum below)
        nc.scalar.activation(ecT[:, sl], psSh[:], AF.Exp,
                             bias=mhat_neg[:], scale=1.0)
        emit_H_chunk()  # PE filler while ACT does pass 3

    # ---- remaining H = C^T tiles [t,d] -> fp32r SBUF ----
    while tH_done[0] < NTC:
        emit_H_chunk()

    # ---- e_c transposes -> [t,j] fp32r chunks ----
    te = [tp_pool.tile([128, 512], F32, tag="tp", name=f"te{b}_{k}")
          for k in range(2)]
    for c in range(NTC):
        nc.tensor.transpose(
            te[c // 4][:, 128 * (c % 4):128 * (c % 4 + 1)].bitcast(F32R),
            ecT[:, 128 * c:128 * (c + 1)], id_r[:])
    ec_sb = mid.tile([128, T], F32R, tag="ecsb")
    for k in range(2):
        nc.vector.tensor_copy(ec_sb[:, 512 * k:512 * (k + 1)], te[k][:])
    csJ = psS_pool.tile([128, 512], F32, tag="psS", bufs=2, name=f"csj{b}")
    for c in range(NTC):
        nc.tensor.matmul(csJ[:, 0:1],
                         ec_sb[:, 128 * c:128 * (c + 1)].bitcast(F32),
                         mcc[:, c:c + 1], start=(c == 0),
                         stop=(c == NTC - 1))
    rc = sm.tile([128, 1], F32, tag="rc")
    nc.vector.reciprocal(rc[:], csJ[:, 0:1])

    # ---- A path first (needs only QT+SqT): outputs start flowing while
    # the q2c chain (interleaved below) finishes for the B path ----
    A_sb = [outp.tile([128, T], F32, tag="asb", name=f"a{b}_{m}")
            for m in range(NCH)]
    HB = [outp.tile([128, T], F32, tag="hbsb", name=f"hb{b}_{m}")
          for m in range(NCH)]
    HA = [outp.tile([128, T], F32, tag="hasb", name=f"ha{b}_{m}")
          for m in range(NCH)]
    psq = psS_pool.tile([128, 512], F32, tag="psS", bufs=2,
                        name=f"psq{b}")
    q2c = mid.tile([128, 512], F32R, tag="q2c")
    for m in range(NCH):
        for h in range(2):
            sl = slice(512 * h, 512 * (h + 1))
            psA = mm_pool.tile([128, 512], F32, tag="mm")
            nc.tensor.matmul(psA[:], QT[:, 128 * m:128 * (m + 1)], SqT[:, sl],
                             start=True, stop=True)
            nc.scalar.activation(A_sb[m][:, sl], psA[:], AF.Copy)
            nc.sync.dma_start(
                out[b, D + 128 * m:D + 128 * (m + 1), sl],
                A_sb[m][:, sl])
        if m == 0:
            # q2c = (1/csum) * sum_c e_c[c].T @ H[c] -> [j,d] fp32r
            for c in range(NTC):
                nc.tensor.matmul(psq[:], ec_sb[:, 128 * c:128 * (c + 1)],
                                 H_sb[c][:], start=(c == 0),
                                 stop=(c == NTC - 1))
            nc.vector.tensor_scalar_mul(q2c[:], psq[:], rc[:])
        nc.gpsimd.tensor_mul(HA[m][:], Cr[m][:], A_sb[m][:])
        if m > 0:
            nc.sync.dma_start(
                out[b, 2 * D + 128 * (m - 1):2 * D + 128 * m, :],
                HA[m - 1][:])
    nc.sync.dma_start(out[b, 2 * D + 128 * (NCH - 1):2 * D + 128 * NCH, :],
                      HA[NCH - 1][:])
    # ---- B path: psB -> H*B fused evacuation, per-half stores ----
    for m in range(NCH):
        for h in range(2):
            sl = slice(512 * h, 512 * (h + 1))
            psB = mm_pool.tile([128, 512], F32, tag="mm")
            nc.tensor.matmul(psB[:], q2c[:, 128 * m:128 * (m + 1)], SqT[:, sl],
                             start=True, stop=True)
            nc.vector.tensor_mul(HB[m][:, sl], Cr[m][:, sl], psB[:])
            nc.sync.dma_start(
                out[b, 3 * D + 128 * m:3 * D + 128 * (m + 1), sl],
                HB[m][:, sl])

    if b == 0 and getattr(nc, "_dbg", None):
        dbg = nc._dbg
        nc.sync.dma_start(dbg["e1"], e1[:].bitcast(F32))
        nc.sync.dma_start(dbg["sqt"], SqT[:].bitcast(F32))
        nc.sync.dma_start(dbg["ect"], ecT[:].bitcast(F32))
        nc.sync.dma_start(dbg["mlog"], mlog[:].bitcast(F32))
        nc.sync.dma_start(dbg["csum"], csum[:])
        nc.sync.dma_start(dbg["rc"], rc[:])
        nc.sync.dma_start(dbg["addc"], addc[:])
        nc.sync.dma_start(dbg["q2c"], q2c[:].bitcast(F32))
        nc.sync.dma_start(dbg["qT"], QT[:].bitcast(F32))
        nc.sync.dma_start(dbg["ecsb"], ec_sb[:].bitcast(F32))
        nc.sync.dma_start(dbg["h0"], H_sb[0][:].bitcast(F32))
        nc.sync.dma_start(dbg["h1"], H_sb[1][:].bitcast(F32))
        nc.sync.dma_start(dbg["qw0"], Qw[0][:])



def _pin_act_table():
    """Pin activation-table selection to 'natural_log_exp_and_others' (the
    one set containing ln+exp+copy+identity together).  The greedy table
    chooser otherwise thrashes between the exp-only and ln-only sets,
    inserting a 1283ns LoadActFuncSet at every Exp<->Ln alternation.
    Entry positions are preserved so act_func_set_id stays a valid index
    into act_info.json."""
    import concourse.bacc as bacc_mod
    if getattr(bacc_mod, "_act_tables_pinned", False):
        return
    orig = bacc_mod.get_activation_tables

    def pinned(arch):
        return {name: (fns if name == "natural_log_exp_and_others" else set())
                for name, fns in orig(arch).items()}

    bacc_mod.get_activation_tables = pinned
    bacc_mod._act_tables_pinned = True


def _build():
    _pin_act_table()
    nc = bacc.Bacc("TRN2", target_bir_lowering=False, debug=False,
                   num_devices=NCORES)
    C = nc.dram_tensor("C", [BPC, D, T], F32, kind="ExternalInput").ap()
    Q = nc.dram_tensor("Q", [BPC, D, J], F32, kind="ExternalInput").ap()
    mask_C = nc.dram_tensor("mask_C", [BPC, T], I32, kind="ExternalInput").ap()
    mask_Q = nc.dram_tensor("mask_Q", [BPC, J], I32, kind="ExternalInput").ap()
    weight = nc.dram_tensor("weight", [3 * D], F32, kind="ExternalInput").ap()
    out = nc.dram_tensor("out", [BPC, 4 * D, T], F32,
                         kind="ExternalOutput").ap()
    dbg = {}
    if DEBUG:
        for nm, shp in [("e1", [128, T]), ("sqt", [128, T]), ("ect", [128, T]),
                        ("mlog", [1, T]), ("csum", [128, 1]), ("rc", [128, 1]),
                        ("addc", [128, 1]), ("q2c", [128, D]), ("qT", [128, D]),
                        ("ecsb", [128, T]), ("h0", [128, D]), ("h1", [128, D]),
                        ("qw0", [128, J])]:
            dbg[nm] = nc.dram_tensor(f"dbg_{nm}", shp, F32,
                                     kind="ExternalOutput").ap()
    nc._dbg = dbg

    with tile.TileContext(nc) as tc:
        import contextlib
        with contextlib.ExitStack() as ctx:
            const = ctx.enter_context(tc.tile_pool(name="const", bufs=1))
            cin = ctx.enter_context(tc.tile_pool(name="cin", bufs=8))
            crp = ctx.enter_context(tc.tile_pool(name="crp", bufs=11))
            qin = ctx.enter_context(tc.tile_pool(name="qin", bufs=4))
            mid = ctx.enter_context(tc.tile_pool(name="mid", bufs=2))
            outp = ctx.enter_context(tc.tile_pool(name="outp", bufs=4))
            sm = ctx.enter_context(tc.tile_pool(name="sm", bufs=4))
            psS_pool = ctx.enter_context(
                tc.tile_pool(name="psS", bufs=2, space="PSUM"))
            tp_pool = ctx.enter_context(
                tc.tile_pool(name="tp", bufs=2, space="PSUM"))
            mm_pool = ctx.enter_context(
                tc.tile_pool(name="mm", bufs=3, space="PSUM"))

            # ---- constants ----
            identity = const.tile([128, 128], F32, tag="identity")
            make_identity(nc, identity[:])
            id_r = const.tile([128, 128], F32R, tag="id_r")
            nc.vector.tensor_copy(id_r[:], identity[:])
            ones128 = const.tile([128, 1], F32, tag="ones128")
            nc.gpsimd.memset(ones128[:], 1.0)
            ones_col_r = const.tile([128, 1], F32R, tag="ones_col_r")
            nc.vector.tensor_copy(ones_col_r[:], ones128[:])
            onesr_f = const.tile([1, 128], F32, tag="onesr_f")
            nc.gpsimd.memset(onesr_f[:], 1.0)
            ones_r32 = const.tile([1, 128], F32R, tag="ones_r32")
            nc.vector.tensor_copy(ones_r32[:], onesr_f[:])
            neg_r32 = const.tile([1, 128], F32R, tag="neg_r32")
            nc.vector.tensor_scalar_mul(neg_r32[:], onesr_f[:], -1.0)
            neg8_r32 = const.tile([1, 128], F32R, tag="neg8_r32")
            nc.vector.tensor_scalar_mul(neg8_r32[:], onesr_f[:], -8.0)
            pos8_r32 = const.tile([1, 128], F32R, tag="pos8_r32")
            nc.vector.tensor_scalar_mul(pos8_r32[:], onesr_f[:], 8.0)
            # weight -> [128, 12]: cols g*4+c hold weight[g*512 + c*128 + p]
            w_col = const.tile([128, 12], F32, tag="w_col")
            nc.sync.dma_start(
                w_col[:], weight.rearrange("(g c p) -> p (g c)", p=128, c=4))
            # mask_Q -> [128, BPC] fp32
            mqi = const.tile([128, BPC], I32, tag="mqi")
            nc.sync.dma_start(mqi[:], mask_Q.rearrange("b j -> j b"))
            mqf_all = const.tile([128, BPC], F32, tag="mqf")
            nc.vector.tensor_copy(mqf_all[:], mqi[:])
            mhat_neg = const.tile([128, 1], F32, tag="mhat")
            nc.gpsimd.memset(mhat_neg[:], -MHAT)
            tiny = const.tile([1, 1], F32, tag="tiny")
            nc.gpsimd.memset(tiny[:], 1e-30)

            consts = (identity, id_r, w_col, ones_col_r, ones_r32,
                      neg_r32, neg8_r32, pos8_r32, mqf_all, mhat_neg,
                      tiny)
            pools = (cin, qin, mid, outp, psS_pool, tp_pool, mm_pool, sm,
                     crp)
            aps = (C, Q, out)

            # software pipeline: batch b+1's input loads are issued on the
            # SP queue BEFORE batch b's compute/stores so the DMA engines
            # never sit idle during the softmax phase.
            L = [_emit_loads(nc, pools, consts, aps, b, mask_C)
                 for b in range(BPC)]
            _emit_round(nc, pools, consts, L[0], 0)
            for b in range(BPC):
                if b + 1 < BPC:
                    _emit_round(nc, pools, consts, L[b + 1], b + 1)
                # H block: direct DRAM->DRAM, no dependencies (the last
                # one is hoisted a cycle early to fill the batch-2 bubble)
                if b < BPC - 1:
                    nc.sync.dma_start(out[b, 0:D, :], C[b, :, :])
                    if b == BPC - 2:
                        nc.sync.dma_start(out[b + 1, 0:D, :], C[b + 1, :, :])
                _emit_batch(nc, tc, pools, consts, aps, b, L[b])
                L[b] = None

    nc.compile()
    return nc


_NC_CACHE = None


def _get_nc():
    global _NC_CACHE
    if _NC_CACHE is None:
        _NC_CACHE = _build()
    return _NC_CACHE


def kernel(C, Q, mask_C, mask_Q, weight):
    nc = _get_nc()
    C = np.ascontiguousarray(C, dtype=np.float32)
    Q = np.ascontiguousarray(Q, dtype=np.float32)
    mask_C = np.ascontiguousarray(mask_C, dtype=np.int32)
    mask_Q = np.ascontiguousarray(mask_Q, dtype=np.int32)
    weight = np.ascontiguousarray(weight, dtype=np.float32)
    in_maps = []
    for c in range(NCORES):
        sl = slice(BPC * c, BPC * (c + 1))
        in_maps.append({
            "C": C[sl], "Q": Q[sl], "mask_C": mask_C[sl],
            "mask_Q": mask_Q[sl], "weight": weight,
        })
    res = bass_utils.run_bass_kernel_spmd(nc, in_maps,
                                          core_ids=list(range(NCORES)))
    return np.concatenate([res.results[c]["out"] for c in range(NCORES)],
                          axis=0)


# revision 32
# speedup vs baseline: 1.0015x; 1.0011x over previous
"""ContextQueryAttention (BiDAF-style) Trainium2 Bass kernel.

Full inputs -> full output; internally data-parallel over batch across 8
NeuronCores (4 batches per core).

Per-batch math (b dropped; C:[d,t], Q:[d,j], d=512, t=1024, j=128):
  H = C^T, U = Q^T
  S[t,j]  = hbias[t] + ubias[j] + sum_d w_hu[d]*C[d,t]*Q[d,j]
  S_q     = softmax_j(mask(S, mask_Q))         # masked -> -1e30
  S_c     = softmax_t(mask(S, mask_C))
  A       = S_q @ U                            # (t,d)
  q2c     = S_c^T @ H                          # (j,d)
  Bmat    = S_q @ q2c                          # (t,d)
  out     = [H; A; H*A; H*Bmat] as (4d, t)

Layout strategy: everything "feature-on-partitions", S^T [j,t] on PE with
hbias folded into the stationary weights (Qw = Q*w_hu + w_h). softmax_j
(partition dim) is done WITHOUT transposes via log-domain normalization:
  e1   = exp(mq*S^T + addc - 100)            (masked-Q, unnormalized)
  cs   = ones^T @ e1                          (PE column sum, [1,t])
  psS += -ones x ln(cs)                      (rank-1, PE)
  SqT  = exp(mq*S^T + addc - 100)            (now exactly softmax_j)
softmax_t is free-dim in the S^T layout with constant max subtraction.
mask_C applied additively via rank-1 fp32r matmul of (m-1)*1e30.
A^T / q2c / Bmat^T as fp32r matmuls (1 cyc/row); outputs written
[C; A^T; C*A^T; C*B^T] in natural d-on-partition layout; H block is a
direct DRAM->DRAM DMA.

DMA scheduling: inputs for batch b+1 are issued on the SP HWDGE queue
BEFORE batch b's compute so the DMA engines stream continuously
(input prefetch fills the S/softmax phase, outputs fill the tail);
A^T outputs go out on the Activation HWDGE queue so a gated store never
head-of-line-blocks the input stream.
"""

import numpy as np

import concourse.bass as bass
import concourse.tile as tile
from concourse import bacc, mybir
from concourse import bass_utils
from concourse.masks import make_identity

F32 = mybir.dt.float32
F32R = mybir.dt.float32r
BF16 = mybir.dt.bfloat16
I32 = mybir.dt.int32

B, T, J, D = 32, 1024, 128, 512
NCORES = 8
BPC = B // NCORES  # batches per core
MHAT = 100.0  # fixed max-subtraction constant for both softmaxes
NCH = D // 128  # 4 d-chunks
NTC = T // 128  # 8 t-chunks

DEBUG = False

AF = mybir.ActivationFunctionType
ALU = mybir.AluOpType


def _emit_loads(nc, pools, consts, aps, b, mask_C):
    """Input DMAs for batch b (SP HWDGE queue)."""
    (cin, qin, mid, outp, psS_pool, tp_pool, mm_pool, sm, crp) = pools
    (C, Q, out) = aps
    Qa = qin.tile([128, NCH * J], F32, tag="qt")
    nc.sync.dma_start(Qa[:].rearrange("p (c j) -> p c j", c=NCH),
                  Q[b].rearrange("(c p) j -> p c j", p=128))
    Qt = [Qa[:, J * i:J * (i + 1)] for i in range(NCH)]
    mci = qin.tile([128, NTC], I32, tag="mci")
    nc.sync.dma_start(mci[:], mask_C[b].rearrange("(c p) -> p c", p=128))
    Ct = []
    for i in range(NCH):
        t = cin.tile([128, T], F32, tag="ct")
        nc.sync.dma_start(t[:], C[b, 128 * i:128 * (i + 1), :])
        Ct.append(t)
    return dict(Ct=Ct, Qt=Qt, mci=mci)


def _emit_round(nc, pools, consts, L, b):
    """One-batch-ahead prep: fp32r rounding of C (Pool, idle during the
    previous batch's softmax), Qw / mask scalars (DVE), ubias (PE).
    Emitting these a batch early keeps them out of the next batch's
    critical path."""
    (cin, qin, mid, outp, psS_pool, tp_pool, mm_pool, sm, crp) = pools
    (identity, id_r, w_col, ones_col_r, ones_r32, neg_r32, neg8_r32,
     pos8_r32, mqf_all, mhat_neg, tiny) = consts
    Qt = L["Qt"]
    Cr = []
    for i in range(NCH):
        t = crp.tile([128, T], F32R, tag="cr", name=f"cr{b}_{i}")
        if b == 0 and i % 2 == 1:
            nc.vector.tensor_copy(t[:], L["Ct"][i][:])
        else:
            nc.gpsimd.tensor_copy(t[:], L["Ct"][i][:])
        Cr.append(t)
    L["Cr"] = Cr
    # mask_C as per-partition columns in the H (t-on-partitions) layout
    mcc = sm.tile([128, NTC], F32, tag="mcc", bufs=4, name=f"mcc{b}")
    nc.vector.tensor_copy(mcc[:], L["mci"][:])
    L["mcc"] = mcc
    # Qw = Q * w_hu + w_h: adding w_h[d] to every column folds
    # hbias[t] = sum_d w_h[d] C[d,t] into the S^T core matmul output.
    Qw = []
    for i in range(NCH):
        t = sm.tile([128, J], F32R, tag="qw", bufs=8, name=f"qw{b}_{i}")
        nc.vector.tensor_scalar_mul(t[:], Qt[i], w_col[:, 8 + i:9 + i])
        nc.vector.tensor_scalar_add(t[:], t[:], w_col[:, i:i + 1])
        Qw.append(t)
    L["Qw"] = Qw
    # ubias[j] = sum_d w_u[d] Q[d,j] : N=1 matmuls -> [j,1]
    ub_ps = psS_pool.tile([128, 512], F32, tag="psS", bufs=2,
                          name=f"ub{b}")
    for i in range(NCH):
        nc.tensor.matmul(ub_ps[:, 0:1], Qt[i], w_col[:, 4 + i:5 + i],
                         start=(i == 0), stop=(i == NCH - 1))
    # mask_Q: scale = mqf, bias = mqf*(ub - MHAT) + (mqf-1)*1e30
    mqf = mqf_all[:, b:b + 1]
    c1 = sm.tile([128, 1], F32, tag="c1")
    nc.vector.tensor_scalar(c1[:], mqf, 1.0e30, -1.0e30,
                            op0=ALU.mult, op1=ALU.add)
    c2 = sm.tile([128, 1], F32, tag="c2")
    nc.vector.tensor_scalar_add(c2[:], ub_ps[:, 0:1], -MHAT)
    nc.vector.tensor_mul(c2[:], c2[:], mqf)
    addc = sm.tile([128, 1], F32, tag="addc")
    nc.vector.tensor_add(addc[:], c1[:], c2[:])
    # 1/8-compressed variants for the range-finding pass
    mq8 = sm.tile([128, 1], F32, tag="mq8")
    nc.vector.tensor_scalar_mul(mq8[:], mqf, 0.125)
    addc8 = sm.tile([128, 1], F32, tag="addc8")
    nc.vector.tensor_scalar_mul(addc8[:], addc[:], 0.125)
    L["addc"] = addc
    L["mq8"] = mq8
    L["addc8"] = addc8


def _emit_batch(nc, tc, pools, consts, aps, b, L):
    """Emit compute + output DMAs for one batch."""
    (cin, qin, mid, outp, psS_pool, tp_pool, mm_pool, sm, crp) = pools
    (identity, id_r, w_col, ones_col_r, ones_r32, neg_r32, neg8_r32,
     pos8_r32, mqf_all, mhat_neg, tiny) = consts
    (C, Q, out) = aps
    Ct, Qt = L["Ct"], L["Qt"]
    Cr, Qw, mcc = L["Cr"], L["Qw"], L["mcc"]
    addc, mq8, addc8 = L["addc"], L["mq8"], L["addc8"]
    mqf = mqf_all[:, b:b + 1]

    # ---- S^T core into PSUM (one bank per t-half), then the two softmax
    # paths.  PE order interleaves the halves so PE works on half 1 while
    # ACT processes half 0. ----
    e1 = mid.tile([128, T], F32R, tag="sqt", bufs=2,
                  name=f"e1_{b}")
    SqT = mid.tile([128, T], F32R, tag="sqt", bufs=2,
                   name=f"sqt{b}")
    ecT = mid.tile([128, T], F32R, tag="ect")

    # H transposes double as PE filler work inside the ACT-bound S chain:
    # emitted between S-chain stages so PE has work while ACT runs.
    H_sb = [None] * NTC
    tH_done = [0]

    def emit_H_chunk():
        c = tH_done[0]
        if c >= NTC:
            return
        tH_done[0] += 1
        tH = tp_pool.tile([128, 512], F32, tag="tp", name=f"tH{b}_{c}")
        for i in range(NCH):
            nc.tensor.transpose(tH[:, 128 * i:128 * (i + 1)].bitcast(F32R),
                                Cr[i][:, 128 * c:128 * (c + 1)], id_r[:])
        hs = mid.tile([128, 512], F32R, tag="hsb", bufs=9,
                      name=f"hsb{b}_{c}")
        nc.vector.tensor_scalar_mul(hs[:], tH[:], mcc[:, c:c + 1])
        H_sb[c] = hs

    for h in range(2):
        sl = slice(512 * h, 512 * (h + 1))
        mlog = sm.tile([1, 512], F32R, tag="mlog", bufs=2,
                       name=f"mlog{b}_{h}")
        ln8 = sm.tile([1, 512], F32R, tag="ln8", bufs=2,
                      name=f"ln8{b}_{h}")
        psSh = psS_pool.tile([128, 512], F32, tag="psS", bufs=2,
                             name=f"psS{b}_{h}")
        for i in range(NCH):
            nc.tensor.matmul(psSh[:], Qw[i][:], Cr[i][:, sl],
                             start=(i == 0), stop=False)
        # pass 0 (range finder): exp((S+ub-100)/8); 8*ln of its column sum
        # is >= the column max of S+ub-100 (within +8*ln(128)), keeping
        # every later exp/ln safely in range.
        nc.scalar.activation(e1[:, sl], psSh[:], AF.Exp,
                             bias=addc8[:], scale=mq8[:])
        emit_H_chunk()  # PE filler while ACT runs
        pcs8 = psS_pool.tile([1, 512], F32, tag="cs", bufs=1,
                             name=f"cs8{b}_{h}")
        nc.tensor.matmul(pcs8[:], ones_col_r[:], e1[:, sl],
                         start=True, stop=True)
        nc.scalar.activation(ln8[:], pcs8[:], AF.Ln, bias=tiny[:])
        nc.tensor.matmul(psSh[:], neg8_r32[:], ln8[:],
                         start=False, stop=False)
        # pass 1: shifted masked-Q exp, values in (0, 1]
        nc.scalar.activation(e1[:, sl], psSh[:], AF.Exp,
                             bias=addc[:], scale=mqf)
        emit_H_chunk()  # PE filler while ACT does pass 1
        # column sum over j (partition dim) on PE -> [1, 512]
        pcs = psS_pool.tile([1, 512], F32, tag="cs", bufs=1,
                            name=f"cs{b}_{h}")
        nc.tensor.matmul(pcs[:], ones_col_r[:], e1[:, sl],
                         start=True, stop=True)
        # +ln(colsum) row; rank-1 subtract into psS
        nc.scalar.activation(mlog[:], pcs[:], AF.Ln, bias=tiny[:])
        emit_H_chunk()  # PE filler while ACT does the Ln
        nc.tensor.matmul(psSh[:], neg_r32[:], mlog[:],
                         start=False, stop=False)
        # pass 2: normalized softmax_j -> SqT
        nc.scalar.activation(SqT[:, sl], psSh[:], AF.Exp,
                             bias=addc[:], scale=mqf)
        emit_H_chunk()  # PE filler while ACT does pass 2
        # undo the shifts, add mask_C row
        nc.tensor.matmul(psSh[:], ones_r32[:], mlog[:],
                         start=False, stop=False)
        nc.tensor.matmul(psSh[:], pos8_r32[:], ln8[:],
                         start=False, stop=True)
        # pass 3: S_c exp with constant max subtraction (mask_C is applied
        # multiplicatively to H, so masked-t terms drop out of q2c and the
        # column sum below)
        nc.scalar.activation(ecT[:, sl], psSh[:], AF.Exp,
                             bias=mhat_neg[:], scale=1.0)
        emit_H_chunk()  # PE filler while ACT does pass 3

    # ---- remaining H = C^T tiles [t,d] -> fp32r SBUF ----
    while tH_done[0] < NTC:
        emit_H_chunk()

    # ---- Q^T [j,d] fp32r ----
    tQ = tp_pool.tile([128, 512], F32, tag="tp")
    for i in range(NCH):
        nc.tensor.transpose(tQ[:, 128 * i:128 * (i + 1)], Qt[i],
                            identity[:])
    QT = mid.tile([128, 512], F32R, tag="qT")
    nc.vector.tensor_copy(QT[:], tQ[:])

    # ---- e_c transposes -> [t,j] fp32r chunks ----
    te = [tp_pool.tile([128, 512], F32, tag="tp", name=f"te{b}_{k}")
          for k in range(2)]
    for c in range(NTC):
        nc.tensor.transpose(
            te[c // 4][:, 128 * (c % 4):128 * (c % 4 + 1)].bitcast(F32R),
            ecT[:, 128 * c:128 * (c + 1)], id_r[:])
    ec_sb = mid.tile([128, T], F32R, tag="ecsb")
    for k in range(2):
        nc.vector.tensor_copy(ec_sb[:, 512 * k:512 * (k + 1)], te[k][:])
    csJ = psS_pool.tile([128, 512], F32, tag="psS", bufs=2, name=f"csj{b}")
    for c in range(NTC):
        nc.tensor.matmul(csJ[:, 0:1],
                         ec_sb[:, 128 * c:128 * (c + 1)].bitcast(F32),
                         mcc[:, c:c + 1], start=(c == 0),
                         stop=(c == NTC - 1))
    rc = sm.tile([128, 1], F32, tag="rc")
    nc.vector.reciprocal(rc[:], csJ[:, 0:1])

    # ---- A path first (needs only QT+SqT): outputs start flowing while
    # the q2c chain (interleaved below) finishes for the B path ----
    A_sb = [outp.tile([128, T], F32, tag="asb", name=f"a{b}_{m}")
            for m in range(NCH)]
    HB = [outp.tile([128, T], F32, tag="hbsb", name=f"hb{b}_{m}")
          for m in range(NCH)]
    HA = [outp.tile([128, T], F32, tag="hasb", name=f"ha{b}_{m}")
          for m in range(NCH)]
    psq = psS_pool.tile([128, 512], F32, tag="psS", bufs=2,
                        name=f"psq{b}")
    q2c = mid.tile([128, 512], F32R, tag="q2c")
    for m in range(NCH):
        for h in range(2):
            sl = slice(512 * h, 512 * (h + 1))
            psA = mm_pool.tile([128, 512], F32, tag="mm")
            nc.tensor.matmul(psA[:], QT[:, 128 * m:128 * (m + 1)], SqT[:, sl],
                             start=True, stop=True)
            nc.scalar.activation(A_sb[m][:, sl], psA[:], AF.Copy)
            nc.sync.dma_start(
                out[b, D + 128 * m:D + 128 * (m + 1), sl],
                A_sb[m][:, sl])
        if m == 0:
            # q2c = (1/csum) * sum_c e_c[c].T @ H[c] -> [j,d] fp32r
            for c in range(NTC):
                nc.tensor.matmul(psq[:], ec_sb[:, 128 * c:128 * (c + 1)],
                                 H_sb[c][:], start=(c == 0),
                                 stop=(c == NTC - 1))
            nc.vector.tensor_scalar_mul(q2c[:], psq[:], rc[:])
        nc.gpsimd.tensor_mul(HA[m][:], Cr[m][:], A_sb[m][:])
        if m > 0:
            nc.sync.dma_start(
                out[b, 2 * D + 128 * (m - 1):2 * D + 128 * m, :],
                HA[m - 1][:])
    nc.sync.dma_start(out[b, 2 * D + 128 * (NCH - 1):2 * D + 128 * NCH, :],
                      HA[NCH - 1][:])
    # ---- B path: psB -> H*B fused evacuation, per-half stores ----
    for m in range(NCH):
        for h in range(2):
            sl = slice(512 * h, 512 * (h + 1))
            psB = mm_pool.tile([128, 512], F32, tag="mm")
            nc.tensor.matmul(psB[:], q2c[:, 128 * m:128 * (m + 1)], SqT[:, sl],
                             start=True, stop=True)
            nc.vector.tensor_mul(HB[m][:, sl], Cr[m][:, sl], psB[:])
            nc.sync.dma_start(
                out[b, 3 * D + 128 * m:3 * D + 128 * (m + 1), sl],
                HB[m][:, sl])

    if b == 0 and getattr(nc, "_dbg", None):
        dbg = nc._dbg
        nc.sync.dma_start(dbg["e1"], e1[:].bitcast(F32))
        nc.sync.dma_start(dbg["sqt"], SqT[:].bitcast(F32))
        nc.sync.dma_start(dbg["ect"], ecT[:].bitcast(F32))
        nc.sync.dma_start(dbg["mlog"], mlog[:].bitcast(F32))
        nc.sync.dma_start(dbg["csum"], csum[:])
        nc.sync.dma_start(dbg["rc"], rc[:])
        nc.sync.dma_start(dbg["addc"], addc[:])
        nc.sync.dma_start(dbg["q2c"], q2c[:].bitcast(F32))
        nc.sync.dma_start(dbg["qT"], QT[:].bitcast(F32))
        nc.sync.dma_start(dbg["ecsb"], ec_sb[:].bitcast(F32))
        nc.sync.dma_start(dbg["h0"], H_sb[0][:].bitcast(F32))
        nc.sync.dma_start(dbg["h1"], H_sb[1][:].bitcast(F32))
        nc.sync.dma_start(dbg["qw0"], Qw[0][:])



def _pin_act_table():
    """Pin activation-table selection to 'natural_log_exp_and_others' (the
    one set containing ln+exp+copy+identity together).  The greedy table
    chooser otherwise thrashes between the exp-only and ln-only sets,
    inserting a 1283ns LoadActFuncSet at every Exp<->Ln alternation.
    Entry positions are preserved so act_func_set_id stays a valid index
    into act_info.json."""
    import concourse.bacc as bacc_mod
    if getattr(bacc_mod, "_act_tables_pinned", False):
        return
    orig = bacc_mod.get_activation_tables

    def pinned(arch):
        return {name: (fns if name == "natural_log_exp_and_others" else set())
                for name, fns in orig(arch).items()}

    bacc_mod.get_activation_tables = pinned
    bacc_mod._act_tables_pinned = True


def _build():
    _pin_act_table()
    nc = bacc.Bacc("TRN2", target_bir_lowering=False, debug=False,
                   num_devices=NCORES)
    C = nc.dram_tensor("C", [BPC, D, T], F32, kind="ExternalInput").ap()
    Q = nc.dram_tensor("Q", [BPC, D, J], F32, kind="ExternalInput").ap()
    mask_C = nc.dram_tensor("mask_C", [BPC, T], I32, kind="ExternalInput").ap()
    mask_Q = nc.dram_tensor("mask_Q", [BPC, J], I32, kind="ExternalInput").ap()
    weight = nc.dram_tensor("weight", [3 * D], F32, kind="ExternalInput").ap()
    out = nc.dram_tensor("out", [BPC, 4 * D, T], F32,
                         kind="ExternalOutput").ap()
    dbg = {}
    if DEBUG:
        for nm, shp in [("e1", [128, T]), ("sqt", [128, T]), ("ect", [128, T]),
                        ("mlog", [1, T]), ("csum", [128, 1]), ("rc", [128, 1]),
                        ("addc", [128, 1]), ("q2c", [128, D]), ("qT", [128, D]),
                        ("ecsb", [128, T]), ("h0", [128, D]), ("h1", [128, D]),
                        ("qw0", [128, J])]:
            dbg[nm] = nc.dram_tensor(f"dbg_{nm}", shp, F32,
                                     kind="ExternalOutput").ap()
    nc._dbg = dbg

    with tile.TileContext(nc) as tc:
        import contextlib
        with contextlib.ExitStack() as ctx:
            const = ctx.enter_context(tc.tile_pool(name="const", bufs=1))
            cin = ctx.enter_context(tc.tile_pool(name="cin", bufs=8))
            crp = ctx.enter_context(tc.tile_pool(name="crp", bufs=11))
            qin = ctx.enter_context(tc.tile_pool(name="qin", bufs=4))
            mid = ctx.enter_context(tc.tile_pool(name="mid", bufs=2))
            outp = ctx.enter_context(tc.tile_pool(name="outp", bufs=4))
            sm = ctx.enter_context(tc.tile_pool(name="sm", bufs=4))
            psS_pool = ctx.enter_context(
                tc.tile_pool(name="psS", bufs=2, space="PSUM"))
            tp_pool = ctx.enter_context(
                tc.tile_pool(name="tp", bufs=2, space="PSUM"))
            mm_pool = ctx.enter_context(
                tc.tile_pool(name="mm", bufs=3, space="PSUM"))

            # ---- constants ----
            identity = const.tile([128, 128], F32, tag="identity")
            make_identity(nc, identity[:])
            id_r = const.tile([128, 128], F32R, tag="id_r")
            nc.vector.tensor_copy(id_r[:], identity[:])
            ones128 = const.tile([128, 1], F32, tag="ones128")
            nc.gpsimd.memset(ones128[:], 1.0)
            ones_col_r = const.tile([128, 1], F32R, tag="ones_col_r")
            nc.vector.tensor_copy(ones_col_r[:], ones128[:])
            onesr_f = const.tile([1, 128], F32, tag="onesr_f")
            nc.gpsimd.memset(onesr_f[:], 1.0)
            ones_r32 = const.tile([1, 128], F32R, tag="ones_r32")
            nc.vector.tensor_copy(ones_r32[:], onesr_f[:])
            neg_r32 = const.tile([1, 128], F32R, tag="neg_r32")
            nc.vector.tensor_scalar_mul(neg_r32[:], onesr_f[:], -1.0)
            neg8_r32 = const.tile([1, 128], F32R, tag="neg8_r32")
            nc.vector.tensor_scalar_mul(neg8_r32[:], onesr_f[:], -8.0)
            pos8_r32 = const.tile([1, 128], F32R, tag="pos8_r32")
            nc.vector.tensor_scalar_mul(pos8_r32[:], onesr_f[:], 8.0)
            # weight -> [128, 12]: cols g*4+c hold weight[g*512 + c*128 + p]
            w_col = const.tile([128, 12], F32, tag="w_col")
            nc.sync.dma_start(
                w_col[:], weight.rearrange("(g c p) -> p (g c)", p=128, c=4))
            # mask_Q -> [128, BPC] fp32
            mqi = const.tile([128, BPC], I32, tag="mqi")
            nc.sync.dma_start(mqi[:], mask_Q.rearrange("b j -> j b"))
            mqf_all = const.tile([128, BPC], F32, tag="mqf")
            nc.vector.tensor_copy(mqf_all[:], mqi[:])
            mhat_neg = const.tile([128, 1], F32, tag="mhat")
            nc.gpsimd.memset(mhat_neg[:], -MHAT)
            tiny = const.tile([1, 1], F32, tag="tiny")
            nc.gpsimd.memset(tiny[:], 1e-30)

            consts = (identity, id_r, w_col, ones_col_r, ones_r32,
                      neg_r32, neg8_r32, pos8_r32, mqf_all, mhat_neg,
                      tiny)
            pools = (cin, qin, mid, outp, psS_pool, tp_pool, mm_pool, sm,
                     crp)
            aps = (C, Q, out)

            # software pipeline: batch b+1's input loads are issued on the
            # SP queue BEFORE batch b's compute/stores so the DMA engines
            # never sit idle during the softmax phase.
            L = [_emit_loads(nc, pools, consts, aps, b, mask_C)
                 for b in range(BPC)]
            _emit_round(nc, pools, consts, L[0], 0)
            for b in range(BPC):
                if b + 1 < BPC:
                    _emit_round(nc, pools, consts, L[b + 1], b + 1)
                # H block: direct DRAM->DRAM, no dependencies (the last
                # one is hoisted a cycle early to fill the batch-2 bubble)
                if b < BPC - 1:
                    nc.sync.dma_start(out[b, 0:D, :], C[b, :, :])
                    if b == BPC - 2:
                        nc.sync.dma_start(out[b + 1, 0:D, :], C[b + 1, :, :])
                _emit_batch(nc, tc, pools, consts, aps, b, L[b])
                L[b] = None

    nc.compile()
    return nc


_NC_CACHE = None


def _get_nc():
    global _NC_CACHE
    if _NC_CACHE is None:
        _NC_CACHE = _build()
    return _NC_CACHE


def kernel(C, Q, mask_C, mask_Q, weight):
    nc = _get_nc()
    C = np.ascontiguousarray(C, dtype=np.float32)
    Q = np.ascontiguousarray(Q, dtype=np.float32)
    mask_C = np.ascontiguousarray(mask_C, dtype=np.int32)
    mask_Q = np.ascontiguousarray(mask_Q, dtype=np.int32)
    weight = np.ascontiguousarray(weight, dtype=np.float32)
    in_maps = []
    for c in range(NCORES):
        sl = slice(BPC * c, BPC * (c + 1))
        in_maps.append({
            "C": C[sl], "Q": Q[sl], "mask_C": mask_C[sl],
            "mask_Q": mask_Q[sl], "weight": weight,
        })
    res = bass_utils.run_bass_kernel_spmd(nc, in_maps,
                                          core_ids=list(range(NCORES)))
    return np.concatenate([res.results[c]["out"] for c in range(NCORES)],
                          axis=0)
